# revision 25
# baseline (speedup 1.0000x reference)
"""Trainium2 Bass kernel for nn_BasicCNN (conv bank + LoRA-masked recurrent net).

Pure data-parallel over batch (DP8) - zero collectives. Each core handles a
128-row batch shard end to end; the cost-model's collective charge (15us +
bytes/40GBps, serialized on COLLECTIVE_CORES) is avoided entirely.

Per-core plan:
 - W1 = (W + 2*(A@B))*mask + I is built on the HOST (free) with the +I fold
   implementing the residual connection. Shipped in four pieces:
     wa8 [1024, 3072] fp8e4  SEN rows x non-O cols     SBUF-resident
     wao [1024, 1024] bf16   SEN rows x O cols         SBUF-resident
     wb  [3072, 1024] bf16   rows 1024:4096 x O cols   SBUF-resident
     ws  [3072, 3072] fp8e4  rows 1024:4096 x cols 0:3072, streamed from HBM
         once per full timestep (t2, t3), fed to the PE directly as the
         stationary operand of mixed-dtype (fp8 x bf16) matmuls.
   fp8 placement was chosen by error-budget probes: quantizing the SEN rows'
   O-cols or the O-col block pushes rel err past 1.3e-2; this split measures
   9.6e-3 on HW vs the 2e-2 gate.
 - State kept transposed [state_dim, batch] so W tiles are stationary and the
   matmul output [m-chunk, batch] is already next step's moving layout; no
   transposes anywhere.
 - k-outer accumulation: PSUM = 8 bank tiles [128,512], 4 accumulator slices
   each (32 live m-chunks; one start/stop per bank exploits the 2KB
   pending-zero region), so the PE consumes W tiles in DMA arrival order.
 - Timestep cost shape: t0 is a bias-add, t1 contracts SEN rows only, t4
   computes O-cols only; t2/t3 are full 4096x4096.
 - Conv bank = one dense matmul vs a host-assembled [512, 3328] scatter of
   the conv kernels.
 - Queues: the big consumption-ordered stream (wbig, ipw, wa, wb, ws) on
   sync/SP; small vectors + x + outw prefetch on gpsimd/Pool so they never
   head-of-line block the stream. Output staged in bf16, written in quarters.

Measured (MultiCoreSim cost model): 181.3us vs 618.0us TP4xDP2 baseline (3.41x).
Breakdown at floor: 47us front (= DMA bytes for conv/ip/W-slab weights; the
last wa k-tile is split by column halves so only 4 matmuls trail the final
arrival), 130us PE-saturated recurrence (bf16 flop floor, gapless), 3.9us
epilogue (last write latency + end barrier; out-proj chains sized 4/4/4/3/1
so the final chain is one m-chunk).
"""
import sys

for _p in ("/opt/trn_rl_repo", "/root/.axon_site/_ro/trn_rl_repo"):
    if _p not in sys.path:
        sys.path.append(_p)

import numpy as np
import ml_dtypes

import concourse.bacc as bacc
import concourse.mybir as mybir
import concourse.tile as tile
from concourse.bass_utils import run_bass_kernel_spmd

dt = mybir.dt
BF16 = ml_dtypes.bfloat16
FP8 = ml_dtypes.float8_e4m3
AF = mybir.ActivationFunctionType

N_CORES = 8
B = 1024
HW = 8
C_IN = 8
FN = 16
SEN, INT, OUT = 1024, 2048, 1024
TOT = 4096
CNN_OUT = 3264
CNN_PAD = 3328
NUM_OUT = 1968
NUM_PAD = 2048
LORA_SCALE = 2.0

BSH = B // N_CORES           # 128 batch rows per core
KT = TOT // 128              # 32 k-tiles of the state dim
AKT = SEN // 128             # 8  k-tiles in the resident SEN slab (wa)
SKT = KT - AKT               # 24 streamed k-tiles (rows 1024:4096)
SCOL = TOT - OUT             # 3072 streamed cols (0:3072)
CM = CNN_PAD // 128          # 26 conv m-chunks
NM = KT                      # 32 recurrence m-chunks
OM0 = SCOL // 128            # 24 = first O m-chunk index
NO = NUM_PAD // 128          # 16 out-proj m-chunks


def _wa_stat(wa8_sb, wao_sb, k, m):
    if m < OM0:
        return wa8_sb[:, k, m * 128:(m + 1) * 128]
    return wao_sb[:, k, (m - OM0) * 128:(m - OM0 + 1) * 128]


def _build_program(reps: int = 1, use_cc: bool = True, upto: int = 99):
    nc = bacc.Bacc("TRN2", target_bir_lowering=False, debug=False,
                   enable_asserts=True, num_devices=N_CORES)

    xT_d = nc.dram_tensor("xT", [512, BSH], dt.bfloat16, kind="ExternalInput")
    wbig_d = nc.dram_tensor("wbig", [512, CNN_PAD], dt.bfloat16, kind="ExternalInput")
    cbias_d = nc.dram_tensor("cbias", [128, CM], dt.float32, kind="ExternalInput")
    ipw_d = nc.dram_tensor("ipw", [CNN_PAD, SEN], dt.bfloat16, kind="ExternalInput")
    ipb_d = nc.dram_tensor("ipb", [128, AKT], dt.float32, kind="ExternalInput")
    wa8_d = nc.dram_tensor("wa8", [SEN, SCOL], dt.float8e4, kind="ExternalInput")
    wao_d = nc.dram_tensor("wao", [SEN, OUT], dt.bfloat16, kind="ExternalInput")
    wb_d = nc.dram_tensor("wb", [TOT - SEN, OUT], dt.bfloat16, kind="ExternalInput")
    ws_d = nc.dram_tensor("ws", [TOT - SEN, SCOL], dt.float8e4, kind="ExternalInput")
    outw_d = nc.dram_tensor("outw", [OUT, NUM_PAD], dt.bfloat16, kind="ExternalInput")
    ob_d = nc.dram_tensor("ob", [128, NO], dt.float32, kind="ExternalInput")

    outT_d = nc.dram_tensor("outT", [NUM_PAD, BSH], dt.bfloat16, kind="ExternalOutput")

    with tile.TileContext(nc) as tc:
        with tc.tile_pool(name="pers", bufs=1) as pers, \
             tc.tile_pool(name="psum", bufs=8, space="PSUM") as psp, \
             tc.tile_pool(name="wsp", bufs=6) as wsp, \
             tc.tile_pool(name="wbigp", bufs=8) as wbp, \
             tc.tile_pool(name="ipwp", bufs=3) as ipp, \
             tc.tile_pool(name="outwp", bufs=1) as owp:

            wa8_sb = pers.tile([128, AKT, SCOL], dt.float8e4, tag="wa8_sb")
            wao_sb = pers.tile([128, AKT, OUT], dt.bfloat16, tag="wao_sb")
            wb_sb = pers.tile([128, SKT, OUT], dt.bfloat16, tag="wb_sb")
            stA = pers.tile([128, KT, BSH], dt.bfloat16, tag="stA")
            stB = pers.tile([128, KT, BSH], dt.bfloat16, tag="stB")
            featT = pers.tile([128, CM, BSH], dt.bfloat16, tag="featT")
            xT_sb = pers.tile([128, 4, BSH], dt.bfloat16, tag="xT_sb")
            cbias_sb = pers.tile([128, CM], dt.float32, tag="cbias_sb")
            ipb_sb = pers.tile([128, AKT], dt.float32, tag="ipb_sb")
            ob_sb = pers.tile([128, NO], dt.float32, tag="ob_sb")
            ostage = pers.tile([128, NO, BSH], dt.bfloat16, tag="ostage")

            nc.gpsimd.dma_start(out=xT_sb[:, :, :],
                                in_=xT_d.rearrange("(k p) b -> p k b", p=128))
            nc.gpsimd.dma_start(out=cbias_sb[:], in_=cbias_d[:, :])
            nc.gpsimd.dma_start(out=ipb_sb[:], in_=ipb_d[:, :])
            nc.gpsimd.dma_start(out=ob_sb[:], in_=ob_d[:, :])

            # resident W slabs: loaded once, k-tile granular so t1 can chase
            # the arrivals
            wa_loads_done = False

            for rep in range(reps):
                # ---- conv bank: featT[m] = relu(wbig[:,m].T @ xT + cb) ----
                wbig_t = []
                for kh in range(8):
                    k, h = divmod(kh, 2)
                    t = wbp.tile([128, CNN_PAD // 2], dt.bfloat16, tag="wbig",
                                 name=f"wbig_t{kh}")
                    if kh == 0:
                        for q in range(2):
                            nc.sync.dma_start(
                                out=t[:, q * 832:(q + 1) * 832],
                                in_=wbig_d[0:128, q * 832:(q + 1) * 832])
                    else:
                        nc.sync.dma_start(
                            out=t[:],
                            in_=wbig_d[k * 128:(k + 1) * 128,
                                       h * (CNN_PAD // 2):(h + 1) * (CNN_PAD // 2)])
                    wbig_t.append(t)
                # 26 m-chunks -> 7 psum groups of <=4
                conv_ps = [psp.tile([128, 512], dt.float32, tag="ps",
                                    name=f"conv_ps{g}") for g in range(7)]
                for k in range(4):
                    for m in range(CM):
                        g, j = divmod(m, 4)
                        last_in_g = m == CM - 1 or j == 3
                        h, mh = divmod(m, CM // 2)
                        nc.tensor.matmul(conv_ps[g][:, j * 128:(j + 1) * 128],
                                         wbig_t[2 * k + h][:, mh * 128:(mh + 1) * 128],
                                         xT_sb[:, k, :],
                                         start=(k == 0 and j == 0),
                                         stop=(k == 3 and last_in_g))
                for m in range(CM):
                    g, j = divmod(m, 4)
                    nc.scalar.activation(featT[:, m, :],
                                         conv_ps[g][:, j * 128:(j + 1) * 128],
                                         AF.Relu, bias=cbias_sb[:, m:m + 1])

                if upto < 2:
                    continue
                # ---- input proj: stA[0:8] = relu(feat @ ipw + ipb) ----
                ip_ps = [psp.tile([128, 512], dt.float32, tag="ps",
                                  name=f"ip_ps{g}") for g in range(2)]
                for k2 in range(CM // 2):
                    ipw_t = ipp.tile([128, 2, SEN], dt.bfloat16, tag="ipw")
                    nc.sync.dma_start(
                        out=ipw_t[:, :, :],
                        in_=ipw_d[k2 * 256:(k2 + 1) * 256, :].rearrange(
                            "(k p) c -> p k c", p=128))
                    for i in range(2):
                        k = 2 * k2 + i
                        for m in range(AKT):
                            g, j = divmod(m, 4)
                            nc.tensor.matmul(ip_ps[g][:, j * 128:(j + 1) * 128],
                                             ipw_t[:, i, m * 128:(m + 1) * 128],
                                             featT[:, k, :],
                                             start=(k == 0 and j == 0),
                                             stop=(k == CM - 1 and j == 3))
                for m in range(AKT):
                    g, j = divmod(m, 4)
                    nc.scalar.activation(stA[:, m, :],
                                         ip_ps[g][:, j * 128:(j + 1) * 128],
                                         AF.Relu, bias=ipb_sb[:, m:m + 1])

                if not wa_loads_done:
                    wa_loads_done = True
                    for k in range(AKT - 1):
                        nc.sync.dma_start(
                            out=wa8_sb[:, k, :], in_=wa8_d[k * 128:(k + 1) * 128, :])
                        nc.sync.dma_start(
                            out=wao_sb[:, k, :], in_=wao_d[k * 128:(k + 1) * 128, :])
                    kl = AKT - 1
                    for h in range(2):
                        nc.sync.dma_start(
                            out=wa8_sb[:, kl, h * 1536:(h + 1) * 1536],
                            in_=wa8_d[kl * 128:(kl + 1) * 128,
                                      h * 1536:(h + 1) * 1536])
                    for h in range(2):
                        nc.sync.dma_start(
                            out=wao_sb[:, kl, h * 512:(h + 1) * 512],
                            in_=wao_d[kl * 128:(kl + 1) * 128,
                                      h * 512:(h + 1) * 512])

                if upto < 3:
                    continue
                # ---- t1: stB[m] = relu(sum_k wa[k, m] stA[k]), k in 0:8 ----
                t1_ps = [psp.tile([128, 512], dt.float32, tag="ps",
                                  name=f"t1_ps{g}") for g in range(8)]
                for k in range(AKT):
                    for m in range(NM):
                        g, j = divmod(m, 4)
                        nc.tensor.matmul(t1_ps[g][:, j * 128:(j + 1) * 128],
                                         _wa_stat(wa8_sb, wao_sb, k, m),
                                         stA[:, k, :],
                                         start=(k == 0 and j == 0),
                                         stop=(k == AKT - 1 and j == 3))
                for g in range(8):
                    nc.scalar.activation(stB[:, 4 * g:4 * g + 4, :], t1_ps[g][:],
                                         AF.Relu)

                if rep == 0:
                    for k in range(SKT):
                        nc.sync.dma_start(
                            out=wb_sb[:, k, :], in_=wb_d[k * 128:(k + 1) * 128, :])

                # ---- t2, t3: full timesteps; src/dst state ping-pong ----
                for t in (2, 3):
                    if upto < t + 2:
                        continue
                    src = stB if t == 2 else stA
                    dst = stA if t == 2 else stB
                    ps = [psp.tile([128, 512], dt.float32, tag="ps",
                                   name=f"t{t}_ps{g}") for g in range(8)]
                    # resident SEN-rows part, all 32 m-chunks
                    for k in range(AKT):
                        for m in range(NM):
                            g, j = divmod(m, 4)
                            nc.tensor.matmul(ps[g][:, j * 128:(j + 1) * 128],
                                             _wa_stat(wa8_sb, wao_sb, k, m),
                                             src[:, k, :],
                                             start=(k == 0 and j == 0), stop=False)
                    # resident O-col part (m 24..31), k 8..31
                    for k in range(SKT):
                        for m in range(OM0, NM):
                            g, j = divmod(m, 4)
                            nc.tensor.matmul(ps[g][:, j * 128:(j + 1) * 128],
                                             wb_sb[:, k, (m - OM0) * 128:
                                                   (m - OM0 + 1) * 128],
                                             src[:, AKT + k, :],
                                             start=False,
                                             stop=(k == SKT - 1 and m % 4 == 3))
                    for g in (6, 7):
                        nc.scalar.activation(dst[:, 4 * g:4 * g + 4, :], ps[g][:],
                                             AF.Relu)
                    # streamed fp8 part (m 0..23), k 8..31, in arrival order
                    for k in range(SKT):
                        ws_t = wsp.tile([128, SCOL], dt.float8e4, tag="ws")
                        nc.sync.dma_start(out=ws_t[:],
                                          in_=ws_d[k * 128:(k + 1) * 128, :])
                        for m in range(OM0):
                            g, j = divmod(m, 4)
                            nc.tensor.matmul(ps[g][:, j * 128:(j + 1) * 128],
                                             ws_t[:, m * 128:(m + 1) * 128],
                                             src[:, AKT + k, :],
                                             start=False,
                                             stop=(k == SKT - 1 and m % 4 == 3))
                    for g in range(6):
                        nc.scalar.activation(dst[:, 4 * g:4 * g + 4, :], ps[g][:],
                                             AF.Relu)

                outw_sb = owp.tile([128, AKT, NUM_PAD], dt.bfloat16, tag="outw")
                for k in range(AKT):
                    nc.gpsimd.dma_start(out=outw_sb[:, k, :],
                                        in_=outw_d[k * 128:(k + 1) * 128, :])

                if upto < 6:
                    continue
                # ---- t4: O-chunks only -> stA[24:32] ----
                def _t4_chain(g):
                    t4_ps = psp.tile([128, 512], dt.float32, tag="ps",
                                     name=f"t4_ps{g}")
                    for k in range(AKT):
                        for j in range(4):
                            mo = 4 * g + j
                            nc.tensor.matmul(t4_ps[:, j * 128:(j + 1) * 128],
                                             wao_sb[:, k, mo * 128:(mo + 1) * 128],
                                             stB[:, k, :],
                                             start=(k == 0 and j == 0), stop=False)
                    for k in range(SKT):
                        for j in range(4):
                            mo = 4 * g + j
                            nc.tensor.matmul(t4_ps[:, j * 128:(j + 1) * 128],
                                             wb_sb[:, k, mo * 128:(mo + 1) * 128],
                                             stB[:, AKT + k, :],
                                             start=False,
                                             stop=(k == SKT - 1 and j == 3))
                    nc.scalar.activation(stA[:, OM0 + 4 * g:OM0 + 4 * g + 4, :],
                                         t4_ps[:], AF.Relu)

                if upto < 7:
                    continue
                # ---- output projection: outT[m] = outw[:,m].T @ O_state + ob ----
                chains = []
                m0 = 0
                for nm_q in (4, 4, 4, 3, 1):
                    out_ps = psp.tile([128, 512], dt.float32, tag="ps",
                                      name=f"out_ps{m0}")
                    chains.append((m0, nm_q, out_ps))
                    m0 += nm_q

                def _out_ksec(ksec):
                    for m0, nm_q, out_ps in chains:
                        for k in ksec:
                            for mi in range(nm_q):
                                nc.tensor.matmul(
                                    out_ps[:, mi * 128:(mi + 1) * 128],
                                    outw_sb[:, k, (m0 + mi) * 128:
                                            (m0 + mi + 1) * 128],
                                    stA[:, OM0 + k, :],
                                    start=(k == 0 and mi == 0),
                                    stop=(k == AKT - 1 and mi == nm_q - 1))

                _t4_chain(0)
                _t4_chain(1)
                _out_ksec(range(AKT))
                for m0, nm_q, out_ps in chains:
                    for mi in range(nm_q):
                        nc.vector.tensor_scalar_add(
                            ostage[:, m0 + mi, :],
                            out_ps[:, mi * 128:(mi + 1) * 128],
                            ob_sb[:, m0 + mi:m0 + mi + 1])
                    nc.sync.dma_start(
                        out=outT_d.rearrange("(m p) b -> p m b",
                                             p=128)[:, m0:m0 + nm_q, :],
                        in_=ostage[:, m0:m0 + nm_q, :])

    nc.compile()
    return nc


_PROGRAM_CACHE: dict = {}


def get_program(reps: int = 1, use_cc: bool = True):
    key = reps
    if key not in _PROGRAM_CACHE:
        _PROGRAM_CACHE[key] = _build_program(reps)
    return _PROGRAM_CACHE[key]


def _assemble_wbig(inputs):
    wbig = np.zeros((512, CNN_PAD), np.float32)
    cbias = np.zeros(CNN_PAD, np.float32)
    off = 0
    for k in range(1, 9):
        o = HW - k + 1
        w = np.asarray(inputs[f"conv_w{k}"], np.float32)
        cb = np.asarray(inputs["conv_b"], np.float32)[k - 1]
        py = np.arange(o)[:, None, None]
        px = np.arange(o)[None, :, None]
        cc = np.arange(C_IN)[None, None, :]
        ncol = np.arange(FN)[:, None, None]
        cols = off + ncol * o * o + py[None, :, :, 0] * o + px[None, :, :, 0]
        for dy in range(k):
            for dx in range(k):
                rows = (py + dy) * 64 + (px + dx) * 8 + cc
                wbig[rows[None, :, :, :], cols[:, :, :, None]] = \
                    w[:, :, dy, dx][:, None, None, :]
        cbias[off + np.arange(FN * o * o)] = np.repeat(cb, o * o)
        off += FN * o * o
    return wbig, cbias


def _vec128(v, cols):
    """Pack a [128*cols] vector as [128, cols] partition-major."""
    out = np.zeros((128, cols), np.float32)
    out[:] = np.asarray(v, np.float32).reshape(cols, 128).T
    return np.ascontiguousarray(out)


def _prep_inputs(inputs):
    x = np.asarray(inputs["x"], np.float32)
    W = np.asarray(inputs["W"], np.float32)
    lora_A = np.asarray(inputs["lora_A"], np.float32)
    lora_B = np.asarray(inputs["lora_B"], np.float32)
    ip_w = np.asarray(inputs["ip_w"], np.float32)
    ip_b = np.asarray(inputs["ip_b"], np.float32)
    out_w = np.asarray(inputs["out_w"], np.float32)
    out_b = np.asarray(inputs["out_b"], np.float32)

    wbig, cbias = _assemble_wbig(inputs)
    ipw_pad = np.zeros((CNN_PAD, SEN), np.float32)
    ipw_pad[:CNN_OUT] = ip_w
    oww_pad = np.zeros((OUT, NUM_PAD), np.float32)
    oww_pad[:, :NUM_OUT] = out_w
    ob_pad = np.zeros(NUM_PAD, np.float32)
    ob_pad[:NUM_OUT] = out_b

    mask = (W != 0).astype(np.float32)
    W1 = (W + (lora_A @ lora_B) * LORA_SCALE) * mask + np.eye(TOT, dtype=np.float32)

    def bf(a):
        return np.ascontiguousarray(a).astype(BF16)

    shared = {
        "wbig": bf(wbig),
        "cbias": _vec128(cbias, CM),
        "ipw": bf(ipw_pad),
        "ipb": _vec128(ip_b, AKT),
        "wa8": np.ascontiguousarray(W1[:SEN, :SCOL]).astype(FP8),
        "wao": bf(W1[:SEN, SCOL:]),
        "wb": bf(W1[SEN:, SCOL:]),
        "ws": np.ascontiguousarray(W1[SEN:, :SCOL]).astype(FP8),
        "outw": bf(oww_pad),
        "ob": _vec128(ob_pad, NO),
    }
    in_maps = []
    for c in range(N_CORES):
        xs = x[c * BSH:(c + 1) * BSH].reshape(BSH, 512).T
        m = dict(shared)
        m["xT"] = bf(xs)
        in_maps.append(m)
    return in_maps


def run_on_hw(in_maps, reps: int = 1):
    nc = get_program(reps)
    return run_bass_kernel_spmd(nc, in_maps, list(range(N_CORES)), trace=False)


def kernel(**inputs) -> np.ndarray:
    in_maps = _prep_inputs(inputs)
    res = run_on_hw(in_maps, reps=1)
    out = np.zeros((B, NUM_PAD), np.float32)
    for c in range(N_CORES):
        out[c * BSH:(c + 1) * BSH, :] = \
            np.asarray(res.results[c]["outT"]).astype(np.float32).T
    return np.ascontiguousarray(out[:, :NUM_OUT])


# revision 27
# speedup vs baseline: 1.0069x; 1.0069x over previous
"""Trainium2 Bass kernel for nn_BasicCNN (conv bank + LoRA-masked recurrent net).

Pure data-parallel over batch (DP8) - zero collectives. Each core handles a
128-row batch shard end to end; the cost-model's collective charge (15us +
bytes/40GBps, serialized on COLLECTIVE_CORES) is avoided entirely.

Per-core plan:
 - W1 = (W + 2*(A@B))*mask + I is built on the HOST (free) with the +I fold
   implementing the residual connection. Shipped in four pieces:
     wa8 [1024, 3072] fp8e4  SEN rows x non-O cols     SBUF-resident
     wao [1024, 1024] bf16   SEN rows x O cols         SBUF-resident
     wb  [3072, 1024] bf16   rows 1024:4096 x O cols   SBUF-resident
     ws  [3072, 3072] fp8e4  rows 1024:4096 x cols 0:3072, streamed from HBM
         once per full timestep (t2, t3), fed to the PE directly as the
         stationary operand of mixed-dtype (fp8 x bf16) matmuls.
   fp8 placement was chosen by error-budget probes: quantizing the SEN rows'
   O-cols or the O-col block pushes rel err past 1.3e-2; this split measures
   9.6e-3 on HW vs the 2e-2 gate.
 - State kept transposed [state_dim, batch] so W tiles are stationary and the
   matmul output [m-chunk, batch] is already next step's moving layout; no
   transposes anywhere.
 - k-outer accumulation: PSUM = 8 bank tiles [128,512], 4 accumulator slices
   each (32 live m-chunks; one start/stop per bank exploits the 2KB
   pending-zero region), so the PE consumes W tiles in DMA arrival order.
 - Timestep cost shape: t0 is a bias-add, t1 contracts SEN rows only, t4
   computes O-cols only; t2/t3 are full 4096x4096.
 - Conv bank = one dense matmul vs a host-assembled [512, 3328] scatter of
   the conv kernels.
 - Queues: the big consumption-ordered stream (wbig, ipw, wa, wb, ws) on
   sync/SP; small vectors + x + outw prefetch on gpsimd/Pool so they never
   head-of-line block the stream. Output staged in bf16, written in quarters.

Measured (MultiCoreSim cost model): 181.3us vs 618.0us TP4xDP2 baseline (3.41x).
Breakdown at floor: 47us front (= DMA bytes for conv/ip/W-slab weights; the
last wa k-tile is split by column halves so only 4 matmuls trail the final
arrival), 130us PE-saturated recurrence (bf16 flop floor, gapless), 3.9us
epilogue (last write latency + end barrier; out-proj chains sized 4/4/4/3/1
so the final chain is one m-chunk).
"""
import sys

for _p in ("/opt/trn_rl_repo", "/root/.axon_site/_ro/trn_rl_repo"):
    if _p not in sys.path:
        sys.path.append(_p)

import numpy as np
import ml_dtypes

import concourse.bacc as bacc
import concourse.mybir as mybir
import concourse.tile as tile
from concourse.bass_utils import run_bass_kernel_spmd

dt = mybir.dt
BF16 = ml_dtypes.bfloat16
FP8 = ml_dtypes.float8_e4m3
AF = mybir.ActivationFunctionType

N_CORES = 8
B = 1024
HW = 8
C_IN = 8
FN = 16
SEN, INT, OUT = 1024, 2048, 1024
TOT = 4096
CNN_OUT = 3264
CNN_PAD = 3328
NUM_OUT = 1968
NUM_PAD = 2048
LORA_SCALE = 2.0

BSH = B // N_CORES           # 128 batch rows per core
KT = TOT // 128              # 32 k-tiles of the state dim
AKT = SEN // 128             # 8  k-tiles in the resident SEN slab (wa)
SKT = KT - AKT               # 24 streamed k-tiles (rows 1024:4096)
SCOL = TOT - OUT             # 3072 streamed cols (0:3072)
CM = CNN_PAD // 128          # 26 conv m-chunks
NM = KT                      # 32 recurrence m-chunks
OM0 = SCOL // 128            # 24 = first O m-chunk index
NO = NUM_PAD // 128          # 16 out-proj m-chunks


def _wa_stat(wa8_sb, wao_sb, k, m):
    if m < OM0:
        return wa8_sb[:, k, m * 128:(m + 1) * 128]
    return wao_sb[:, k, (m - OM0) * 128:(m - OM0 + 1) * 128]


def _build_program(reps: int = 1, use_cc: bool = True, upto: int = 99):
    nc = bacc.Bacc("TRN2", target_bir_lowering=False, debug=False,
                   enable_asserts=True, num_devices=N_CORES)

    xT_d = nc.dram_tensor("xT", [512, BSH], dt.bfloat16, kind="ExternalInput")
    wbig_d = nc.dram_tensor("wbig", [512, CNN_PAD], dt.bfloat16, kind="ExternalInput")
    cbias_d = nc.dram_tensor("cbias", [128, CM], dt.float32, kind="ExternalInput")
    ipw_d = nc.dram_tensor("ipw", [CNN_PAD, SEN], dt.bfloat16, kind="ExternalInput")
    ipb_d = nc.dram_tensor("ipb", [128, AKT], dt.float32, kind="ExternalInput")
    wa8_d = nc.dram_tensor("wa8", [SEN, SCOL], dt.float8e4, kind="ExternalInput")
    wao_d = nc.dram_tensor("wao", [SEN, OUT], dt.bfloat16, kind="ExternalInput")
    wb_d = nc.dram_tensor("wb", [TOT - SEN, OUT], dt.bfloat16, kind="ExternalInput")
    ws_d = nc.dram_tensor("ws", [TOT - SEN, SCOL], dt.float8e4, kind="ExternalInput")
    outw_d = nc.dram_tensor("outw", [OUT, NUM_PAD], dt.bfloat16, kind="ExternalInput")
    ob_d = nc.dram_tensor("ob", [128, NO], dt.float32, kind="ExternalInput")

    outT_d = nc.dram_tensor("outT", [NUM_PAD, BSH], dt.bfloat16, kind="ExternalOutput")

    with tile.TileContext(nc) as tc:
        with tc.tile_pool(name="pers", bufs=1) as pers, \
             tc.tile_pool(name="psum", bufs=8, space="PSUM") as psp, \
             tc.tile_pool(name="wsp", bufs=3) as wsp, \
             tc.tile_pool(name="wbigp", bufs=8) as wbp, \
             tc.tile_pool(name="ipwp", bufs=2) as ipp, \
             tc.tile_pool(name="outwp", bufs=1) as owp, \
             tc.tile_pool(name="st8p", bufs=2) as s8p:

            wa8_sb = pers.tile([128, AKT, SCOL], dt.float8e4, tag="wa8_sb")
            wao_sb = pers.tile([128, AKT, OUT], dt.bfloat16, tag="wao_sb")
            wb_sb = pers.tile([128, SKT, OUT], dt.bfloat16, tag="wb_sb")
            stA = pers.tile([128, KT, BSH], dt.bfloat16, tag="stA")
            stB = pers.tile([128, KT, BSH], dt.bfloat16, tag="stB")
            featT = pers.tile([128, CM, BSH], dt.bfloat16, tag="featT")
            xT_sb = pers.tile([128, 4, BSH], dt.bfloat16, tag="xT_sb")
            cbias_sb = pers.tile([128, CM], dt.float32, tag="cbias_sb")
            ipb_sb = pers.tile([128, AKT], dt.float32, tag="ipb_sb")
            ob_sb = pers.tile([128, NO], dt.float32, tag="ob_sb")
            ostage = pers.tile([128, NO, BSH], dt.bfloat16, tag="ostage")

            nc.gpsimd.dma_start(out=xT_sb[:, :, :],
                                in_=xT_d.rearrange("(k p) b -> p k b", p=128))
            nc.gpsimd.dma_start(out=cbias_sb[:], in_=cbias_d[:, :])
            nc.gpsimd.dma_start(out=ipb_sb[:], in_=ipb_d[:, :])
            nc.gpsimd.dma_start(out=ob_sb[:], in_=ob_d[:, :])

            # resident W slabs: loaded once, k-tile granular so t1 can chase
            # the arrivals
            wa_loads_done = False

            for rep in range(reps):
                # ---- conv bank: featT[m] = relu(wbig[:,m].T @ xT + cb) ----
                wbig_t = []
                for kh in range(8):
                    k, h = divmod(kh, 2)
                    t = wbp.tile([128, CNN_PAD // 2], dt.bfloat16, tag="wbig",
                                 name=f"wbig_t{kh}")
                    if kh == 0:
                        for q in range(2):
                            nc.sync.dma_start(
                                out=t[:, q * 832:(q + 1) * 832],
                                in_=wbig_d[0:128, q * 832:(q + 1) * 832])
                    else:
                        nc.sync.dma_start(
                            out=t[:],
                            in_=wbig_d[k * 128:(k + 1) * 128,
                                       h * (CNN_PAD // 2):(h + 1) * (CNN_PAD // 2)])
                    wbig_t.append(t)
                # 26 m-chunks -> 7 psum groups of <=4
                conv_ps = [psp.tile([128, 512], dt.float32, tag="ps",
                                    name=f"conv_ps{g}") for g in range(7)]
                for k in range(4):
                    for m in range(CM):
                        g, j = divmod(m, 4)
                        last_in_g = m == CM - 1 or j == 3
                        h, mh = divmod(m, CM // 2)
                        nc.tensor.matmul(conv_ps[g][:, j * 128:(j + 1) * 128],
                                         wbig_t[2 * k + h][:, mh * 128:(mh + 1) * 128],
                                         xT_sb[:, k, :],
                                         start=(k == 0 and j == 0),
                                         stop=(k == 3 and last_in_g))
                for m in range(CM):
                    g, j = divmod(m, 4)
                    nc.scalar.activation(featT[:, m, :],
                                         conv_ps[g][:, j * 128:(j + 1) * 128],
                                         AF.Relu, bias=cbias_sb[:, m:m + 1])

                if upto < 2:
                    continue
                # ---- input proj: stA[0:8] = relu(feat @ ipw + ipb) ----
                ip_ps = [psp.tile([128, 512], dt.float32, tag="ps",
                                  name=f"ip_ps{g}") for g in range(2)]
                for k2 in range(CM // 2):
                    ipw_t = ipp.tile([128, 2, SEN], dt.bfloat16, tag="ipw")
                    nc.sync.dma_start(
                        out=ipw_t[:, :, :],
                        in_=ipw_d[k2 * 256:(k2 + 1) * 256, :].rearrange(
                            "(k p) c -> p k c", p=128))
                    for i in range(2):
                        k = 2 * k2 + i
                        for m in range(AKT):
                            g, j = divmod(m, 4)
                            nc.tensor.matmul(ip_ps[g][:, j * 128:(j + 1) * 128],
                                             ipw_t[:, i, m * 128:(m + 1) * 128],
                                             featT[:, k, :],
                                             start=(k == 0 and j == 0),
                                             stop=(k == CM - 1 and j == 3))
                for m in range(AKT):
                    g, j = divmod(m, 4)
                    nc.scalar.activation(stA[:, m, :],
                                         ip_ps[g][:, j * 128:(j + 1) * 128],
                                         AF.Relu, bias=ipb_sb[:, m:m + 1])

                if not wa_loads_done:
                    wa_loads_done = True
                    for k in range(AKT - 1):
                        nc.sync.dma_start(
                            out=wa8_sb[:, k, :], in_=wa8_d[k * 128:(k + 1) * 128, :])
                        nc.sync.dma_start(
                            out=wao_sb[:, k, :], in_=wao_d[k * 128:(k + 1) * 128, :])
                    kl = AKT - 1
                    for h in range(2):
                        nc.sync.dma_start(
                            out=wa8_sb[:, kl, h * 1536:(h + 1) * 1536],
                            in_=wa8_d[kl * 128:(kl + 1) * 128,
                                      h * 1536:(h + 1) * 1536])
                    for h in range(2):
                        nc.sync.dma_start(
                            out=wao_sb[:, kl, h * 512:(h + 1) * 512],
                            in_=wao_d[kl * 128:(kl + 1) * 128,
                                      h * 512:(h + 1) * 512])

                if upto < 3:
                    continue
                # ---- t1: stB[m] = relu(sum_k wa[k, m] stA[k]), k in 0:8 ----
                t1_ps = [psp.tile([128, 512], dt.float32, tag="ps",
                                  name=f"t1_ps{g}") for g in range(8)]
                for k in range(AKT):
                    for m in range(NM):
                        g, j = divmod(m, 4)
                        nc.tensor.matmul(t1_ps[g][:, j * 128:(j + 1) * 128],
                                         _wa_stat(wa8_sb, wao_sb, k, m),
                                         stA[:, k, :],
                                         start=(k == 0 and j == 0),
                                         stop=(k == AKT - 1 and j == 3))
                for g in range(8):
                    nc.scalar.activation(stB[:, 4 * g:4 * g + 4, :], t1_ps[g][:],
                                         AF.Relu)

                if rep == 0:
                    for k in range(SKT):
                        nc.sync.dma_start(
                            out=wb_sb[:, k, :], in_=wb_d[k * 128:(k + 1) * 128, :])

                # ---- t2, t3: full timesteps; src/dst state ping-pong ----
                for t in (2, 3):
                    if upto < t + 2:
                        continue
                    src = stB if t == 2 else stA
                    dst = stA if t == 2 else stB
                    ps = [psp.tile([128, 512], dt.float32, tag="ps",
                                   name=f"t{t}_ps{g}") for g in range(8)]
                    # resident SEN-rows part, all 32 m-chunks
                    for k in range(AKT):
                        for m in range(NM):
                            g, j = divmod(m, 4)
                            nc.tensor.matmul(ps[g][:, j * 128:(j + 1) * 128],
                                             _wa_stat(wa8_sb, wao_sb, k, m),
                                             src[:, k, :],
                                             start=(k == 0 and j == 0), stop=False)
                    # resident O-col part (m 24..31), k 8..31
                    for k in range(SKT):
                        for m in range(OM0, NM):
                            g, j = divmod(m, 4)
                            nc.tensor.matmul(ps[g][:, j * 128:(j + 1) * 128],
                                             wb_sb[:, k, (m - OM0) * 128:
                                                   (m - OM0 + 1) * 128],
                                             src[:, AKT + k, :],
                                             start=False,
                                             stop=(k == SKT - 1 and m % 4 == 3))
                    for g in (6, 7):
                        nc.scalar.activation(dst[:, 4 * g:4 * g + 4, :], ps[g][:],
                                             AF.Relu)
                    # streamed fp8 part (m 0..23), k-pairs, DoubleRow:
                    # both operands fp8, 256-deep contraction per instruction.
                    # Moving state is an fp8 copy of the (bf16) carried state,
                    # used only for this contraction.
                    st8 = s8p.tile([128, SKT // 2, 2, BSH], dt.float8e4,
                                   tag="st8")
                    for k in range(SKT):
                        kp, i = divmod(k, 2)
                        nc.vector.tensor_scalar_max(st8[:, kp, i, :],
                                                    src[:, AKT + k, :], 0.0)
                    for kp in range(SKT // 2):
                        ws_t = wsp.tile([128, 2, SCOL], dt.float8e4, tag="ws")
                        nc.sync.dma_start(
                            out=ws_t[:, :, :],
                            in_=ws_d[kp * 256:(kp + 1) * 256, :].rearrange(
                                "(i p) c -> p i c", p=128))
                        for m in range(OM0):
                            g, j = divmod(m, 4)
                            nc.tensor.matmul(
                                ps[g][:, j * 128:(j + 1) * 128],
                                ws_t[:, :, m * 128:(m + 1) * 128],
                                st8[:, kp, :, :],
                                start=False,
                                stop=(kp == SKT // 2 - 1 and m % 4 == 3),
                                perf_mode=mybir.MatmulPerfMode.DoubleRow)
                    for g in range(6):
                        nc.scalar.activation(dst[:, 4 * g:4 * g + 4, :], ps[g][:],
                                             AF.Relu)

                outw_sb = owp.tile([128, AKT, NUM_PAD], dt.bfloat16, tag="outw")
                for k in range(AKT):
                    nc.gpsimd.dma_start(out=outw_sb[:, k, :],
                                        in_=outw_d[k * 128:(k + 1) * 128, :])

                if upto < 6:
                    continue
                # ---- t4: O-chunks only -> stA[24:32] ----
                def _t4_chain(g):
                    t4_ps = psp.tile([128, 512], dt.float32, tag="ps",
                                     name=f"t4_ps{g}")
                    for k in range(AKT):
                        for j in range(4):
                            mo = 4 * g + j
                            nc.tensor.matmul(t4_ps[:, j * 128:(j + 1) * 128],
                                             wao_sb[:, k, mo * 128:(mo + 1) * 128],
                                             stB[:, k, :],
                                             start=(k == 0 and j == 0), stop=False)
                    for k in range(SKT):
                        for j in range(4):
                            mo = 4 * g + j
                            nc.tensor.matmul(t4_ps[:, j * 128:(j + 1) * 128],
                                             wb_sb[:, k, mo * 128:(mo + 1) * 128],
                                             stB[:, AKT + k, :],
                                             start=False,
                                             stop=(k == SKT - 1 and j == 3))
                    nc.scalar.activation(stA[:, OM0 + 4 * g:OM0 + 4 * g + 4, :],
                                         t4_ps[:], AF.Relu)

                if upto < 7:
                    continue
                # ---- output projection: outT[m] = outw[:,m].T @ O_state + ob ----
                chains = []
                m0 = 0
                for nm_q in (4, 4, 4, 3, 1):
                    out_ps = psp.tile([128, 512], dt.float32, tag="ps",
                                      name=f"out_ps{m0}")
                    chains.append((m0, nm_q, out_ps))
                    m0 += nm_q

                def _out_ksec(ksec):
                    for m0, nm_q, out_ps in chains:
                        for k in ksec:
                            for mi in range(nm_q):
                                nc.tensor.matmul(
                                    out_ps[:, mi * 128:(mi + 1) * 128],
                                    outw_sb[:, k, (m0 + mi) * 128:
                                            (m0 + mi + 1) * 128],
                                    stA[:, OM0 + k, :],
                                    start=(k == 0 and mi == 0),
                                    stop=(k == AKT - 1 and mi == nm_q - 1))

                _t4_chain(0)
                _t4_chain(1)
                _out_ksec(range(AKT))
                for m0, nm_q, out_ps in chains:
                    for mi in range(nm_q):
                        nc.vector.tensor_scalar_add(
                            ostage[:, m0 + mi, :],
                            out_ps[:, mi * 128:(mi + 1) * 128],
                            ob_sb[:, m0 + mi:m0 + mi + 1])
                    nc.sync.dma_start(
                        out=outT_d.rearrange("(m p) b -> p m b",
                                             p=128)[:, m0:m0 + nm_q, :],
                        in_=ostage[:, m0:m0 + nm_q, :])

    nc.compile()
    return nc


_PROGRAM_CACHE: dict = {}


def get_program(reps: int = 1, use_cc: bool = True):
    key = reps
    if key not in _PROGRAM_CACHE:
        _PROGRAM_CACHE[key] = _build_program(reps)
    return _PROGRAM_CACHE[key]


def _assemble_wbig(inputs):
    wbig = np.zeros((512, CNN_PAD), np.float32)
    cbias = np.zeros(CNN_PAD, np.float32)
    off = 0
    for k in range(1, 9):
        o = HW - k + 1
        w = np.asarray(inputs[f"conv_w{k}"], np.float32)
        cb = np.asarray(inputs["conv_b"], np.float32)[k - 1]
        py = np.arange(o)[:, None, None]
        px = np.arange(o)[None, :, None]
        cc = np.arange(C_IN)[None, None, :]
        ncol = np.arange(FN)[:, None, None]
        cols = off + ncol * o * o + py[None, :, :, 0] * o + px[None, :, :, 0]
        for dy in range(k):
            for dx in range(k):
                rows = (py + dy) * 64 + (px + dx) * 8 + cc
                wbig[rows[None, :, :, :], cols[:, :, :, None]] = \
                    w[:, :, dy, dx][:, None, None, :]
        cbias[off + np.arange(FN * o * o)] = np.repeat(cb, o * o)
        off += FN * o * o
    return wbig, cbias


def _vec128(v, cols):
    """Pack a [128*cols] vector as [128, cols] partition-major."""
    out = np.zeros((128, cols), np.float32)
    out[:] = np.asarray(v, np.float32).reshape(cols, 128).T
    return np.ascontiguousarray(out)


def _prep_inputs(inputs):
    x = np.asarray(inputs["x"], np.float32)
    W = np.asarray(inputs["W"], np.float32)
    lora_A = np.asarray(inputs["lora_A"], np.float32)
    lora_B = np.asarray(inputs["lora_B"], np.float32)
    ip_w = np.asarray(inputs["ip_w"], np.float32)
    ip_b = np.asarray(inputs["ip_b"], np.float32)
    out_w = np.asarray(inputs["out_w"], np.float32)
    out_b = np.asarray(inputs["out_b"], np.float32)

    wbig, cbias = _assemble_wbig(inputs)
    ipw_pad = np.zeros((CNN_PAD, SEN), np.float32)
    ipw_pad[:CNN_OUT] = ip_w
    oww_pad = np.zeros((OUT, NUM_PAD), np.float32)
    oww_pad[:, :NUM_OUT] = out_w
    ob_pad = np.zeros(NUM_PAD, np.float32)
    ob_pad[:NUM_OUT] = out_b

    mask = (W != 0).astype(np.float32)
    W1 = (W + (lora_A @ lora_B) * LORA_SCALE) * mask + np.eye(TOT, dtype=np.float32)

    def bf(a):
        return np.ascontiguousarray(a).astype(BF16)

    shared = {
        "wbig": bf(wbig),
        "cbias": _vec128(cbias, CM),
        "ipw": bf(ipw_pad),
        "ipb": _vec128(ip_b, AKT),
        "wa8": np.ascontiguousarray(W1[:SEN, :SCOL]).astype(FP8),
        "wao": bf(W1[:SEN, SCOL:]),
        "wb": bf(W1[SEN:, SCOL:]),
        "ws": np.ascontiguousarray(W1[SEN:, :SCOL]).astype(FP8),
        "outw": bf(oww_pad),
        "ob": _vec128(ob_pad, NO),
    }
    in_maps = []
    for c in range(N_CORES):
        xs = x[c * BSH:(c + 1) * BSH].reshape(BSH, 512).T
        m = dict(shared)
        m["xT"] = bf(xs)
        in_maps.append(m)
    return in_maps


def run_on_hw(in_maps, reps: int = 1):
    nc = get_program(reps)
    return run_bass_kernel_spmd(nc, in_maps, list(range(N_CORES)), trace=False)


def kernel(**inputs) -> np.ndarray:
    in_maps = _prep_inputs(inputs)
    res = run_on_hw(in_maps, reps=1)
    out = np.zeros((B, NUM_PAD), np.float32)
    for c in range(N_CORES):
        out[c * BSH:(c + 1) * BSH, :] = \
            np.asarray(res.results[c]["outT"]).astype(np.float32).T
    return np.ascontiguousarray(out[:, :NUM_OUT])


# revision 35
# speedup vs baseline: 1.2320x; 1.2236x over previous
"""Trainium2 Bass kernel for nn_BasicCNN (conv bank + LoRA-masked recurrent net).

Pure data-parallel over batch (DP8) - zero collectives. Each core handles a
128-row batch shard end to end; the cost-model's collective charge (15us +
bytes/40GBps, serialized on COLLECTIVE_CORES) is avoided entirely.

Per-core plan:
 - W1 = (W + 2*(A@B))*mask + I is built on the HOST (free) with the +I fold
   implementing the residual connection. Shipped in four pieces:
     wa8 [1024, 3072] fp8e4  SEN rows x non-O cols     SBUF-resident
     wao [1024, 1024] bf16   SEN rows x O cols         SBUF-resident
     wb  [3072, 1024] bf16   rows 1024:4096 x O cols   SBUF-resident
     ws  [3072, 3072] fp8e4  rows 1024:4096 x cols 0:3072, streamed from HBM
         once per full timestep (t2, t3), fed to the PE directly as the
         stationary operand of mixed-dtype (fp8 x bf16) matmuls.
   fp8 placement was chosen by error-budget probes: quantizing the SEN rows'
   O-cols or the O-col block pushes rel err past 1.3e-2; this split measures
   9.6e-3 on HW vs the 2e-2 gate.
 - State kept transposed [state_dim, batch] so W tiles are stationary and the
   matmul output [m-chunk, batch] is already next step's moving layout; no
   transposes anywhere.
 - k-outer accumulation: PSUM = 8 bank tiles [128,512], 4 accumulator slices
   each (32 live m-chunks; one start/stop per bank exploits the 2KB
   pending-zero region), so the PE consumes W tiles in DMA arrival order.
 - Timestep cost shape: t0 is a bias-add, t1 contracts SEN rows only, t4
   computes O-cols only; t2/t3 are full 4096x4096.
 - Conv bank = one dense matmul vs a host-assembled [512, 3328] scatter of
   the conv kernels.
 - Queues: the big consumption-ordered stream (wbig, ipw, wa, wb, ws) on
   sync/SP; small vectors + x + outw prefetch on gpsimd/Pool so they never
   head-of-line block the stream. Output staged in bf16, written in quarters.

Measured (MultiCoreSim cost model): 181.3us vs 618.0us TP4xDP2 baseline (3.41x).
Breakdown at floor: 47us front (= DMA bytes for conv/ip/W-slab weights; the
last wa k-tile is split by column halves so only 4 matmuls trail the final
arrival), 130us PE-saturated recurrence (bf16 flop floor, gapless), 3.9us
epilogue (last write latency + end barrier; out-proj chains sized 4/4/4/3/1
so the final chain is one m-chunk).
"""
import sys

for _p in ("/opt/trn_rl_repo", "/root/.axon_site/_ro/trn_rl_repo"):
    if _p not in sys.path:
        sys.path.append(_p)

import numpy as np
import ml_dtypes

import concourse.bacc as bacc
import concourse.mybir as mybir
import concourse.tile as tile
from concourse.bass_utils import run_bass_kernel_spmd

dt = mybir.dt
BF16 = ml_dtypes.bfloat16
FP8 = ml_dtypes.float8_e4m3
AF = mybir.ActivationFunctionType

N_CORES = 8
B = 1024
HW = 8
C_IN = 8
FN = 16
SEN, INT, OUT = 1024, 2048, 1024
TOT = 4096
CNN_OUT = 3264
CNN_PAD = 3328
NUM_OUT = 1968
NUM_PAD = 2048
LORA_SCALE = 2.0

BSH = B // N_CORES           # 128 batch rows per core
KT = TOT // 128              # 32 k-tiles of the state dim
AKT = SEN // 128             # 8  k-tiles in the resident SEN slab (wa)
SKT = KT - AKT               # 24 streamed k-tiles (rows 1024:4096)
SCOL = TOT - OUT             # 3072 streamed cols (0:3072)
CM = CNN_PAD // 128          # 26 conv m-chunks
NM = KT                      # 32 recurrence m-chunks
OM0 = SCOL // 128            # 24 = first O m-chunk index
NO = NUM_PAD // 128          # 16 out-proj m-chunks


def _wa_stat(wa8_sb, wao_sb, k, m):
    if m < OM0:
        return wa8_sb[:, k, m * 128:(m + 1) * 128]
    return wao_sb[:, k, (m - OM0) * 128:(m - OM0 + 1) * 128]


def _build_program(reps: int = 1, use_cc: bool = True, upto: int = 99):
    nc = bacc.Bacc("TRN2", target_bir_lowering=False, debug=False,
                   enable_asserts=True, num_devices=N_CORES)

    xT_d = nc.dram_tensor("xT", [512, BSH], dt.bfloat16, kind="ExternalInput")
    wbig_d = nc.dram_tensor("wbig", [512, CNN_PAD], dt.bfloat16, kind="ExternalInput")
    cbias_d = nc.dram_tensor("cbias", [128, CM], dt.float32, kind="ExternalInput")
    ipw_d = nc.dram_tensor("ipw", [CNN_PAD, SEN], dt.bfloat16, kind="ExternalInput")
    ipb_d = nc.dram_tensor("ipb", [128, AKT], dt.float32, kind="ExternalInput")
    wa8_d = nc.dram_tensor("wa8", [SEN, SCOL], dt.float8e4, kind="ExternalInput")
    wao_d = nc.dram_tensor("wao", [SEN, OUT], dt.bfloat16, kind="ExternalInput")
    wb_d = nc.dram_tensor("wb", [TOT - SEN, OUT], dt.bfloat16, kind="ExternalInput")
    ws_d = nc.dram_tensor("ws", [TOT - SEN, SCOL], dt.float8e4, kind="ExternalInput")
    outw_d = nc.dram_tensor("outw", [OUT, NUM_PAD], dt.bfloat16, kind="ExternalInput")
    ob_d = nc.dram_tensor("ob", [128, NO], dt.float32, kind="ExternalInput")

    outT_d = nc.dram_tensor("outT", [NUM_PAD, BSH], dt.bfloat16, kind="ExternalOutput")

    with tile.TileContext(nc) as tc:
        with tc.tile_pool(name="pers", bufs=1) as pers, \
             tc.tile_pool(name="psum", bufs=8, space="PSUM") as psp, \
             tc.tile_pool(name="wsp", bufs=3) as wsp, \
             tc.tile_pool(name="wbigp", bufs=4) as wbp, \
             tc.tile_pool(name="ipwp", bufs=2) as ipp, \
             tc.tile_pool(name="outwp", bufs=1) as owp, \
             tc.tile_pool(name="st8p", bufs=1) as s8p:

            wa8_sb = pers.tile([128, AKT, SCOL], dt.float8e4, tag="wa8_sb")
            wao_sb = pers.tile([128, AKT, OUT], dt.bfloat16, tag="wao_sb")
            wb_sb = pers.tile([128, SKT, OUT], dt.bfloat16, tag="wb_sb")
            stA = pers.tile([128, KT, BSH], dt.bfloat16, tag="stA")
            stB = pers.tile([128, KT, BSH], dt.bfloat16, tag="stB")
            featT = pers.tile([128, CM, BSH], dt.bfloat16, tag="featT")
            xT_sb = pers.tile([128, 4, BSH], dt.bfloat16, tag="xT_sb")
            cbias_sb = pers.tile([128, CM], dt.float32, tag="cbias_sb")
            ipb_sb = pers.tile([128, AKT], dt.float32, tag="ipb_sb")
            ob_sb = pers.tile([128, NO], dt.float32, tag="ob_sb")
            ostage = pers.tile([128, NO, BSH], dt.bfloat16, tag="ostage")
            ws_c = pers.tile([128, 2, SCOL], dt.float8e4, tag="ws_c")
            ws_c1 = pers.tile([128, 2, SCOL], dt.float8e4, tag="ws_c1")

            nc.gpsimd.dma_start(out=xT_sb[:, :, :],
                                in_=xT_d.rearrange("(k p) b -> p k b", p=128))
            nc.gpsimd.dma_start(out=cbias_sb[:], in_=cbias_d[:, :])
            nc.gpsimd.dma_start(out=ipb_sb[:], in_=ipb_d[:, :])
            nc.gpsimd.dma_start(out=ob_sb[:], in_=ob_d[:, :])

            # resident W slabs: loaded once, k-tile granular so t1 can chase
            # the arrivals
            wa_loads_done = False

            for rep in range(reps):
                # ---- conv bank: featT[m] = relu(wbig[:,m].T @ xT + cb) ----
                wbig_t = []
                for kh in range(8):
                    k, h = divmod(kh, 2)
                    t = wbp.tile([128, CNN_PAD // 2], dt.bfloat16, tag="wbig",
                                 name=f"wbig_t{kh}")
                    if kh == 0:
                        for q in range(2):
                            nc.sync.dma_start(
                                out=t[:, q * 832:(q + 1) * 832],
                                in_=wbig_d[0:128, q * 832:(q + 1) * 832])
                    else:
                        nc.sync.dma_start(
                            out=t[:],
                            in_=wbig_d[k * 128:(k + 1) * 128,
                                       h * (CNN_PAD // 2):(h + 1) * (CNN_PAD // 2)])
                    wbig_t.append(t)
                # 26 m-chunks -> 7 psum groups of <=4
                conv_ps = [psp.tile([128, 512], dt.float32, tag="ps",
                                    name=f"conv_ps{g}") for g in range(7)]
                for k in range(4):
                    for m in range(CM):
                        g, j = divmod(m, 4)
                        last_in_g = m == CM - 1 or j == 3
                        h, mh = divmod(m, CM // 2)
                        nc.tensor.matmul(conv_ps[g][:, j * 128:(j + 1) * 128],
                                         wbig_t[2 * k + h][:, mh * 128:(mh + 1) * 128],
                                         xT_sb[:, k, :],
                                         start=(k == 0 and j == 0),
                                         stop=(k == 3 and last_in_g))
                for m in range(CM):
                    g, j = divmod(m, 4)
                    nc.scalar.activation(featT[:, m, :],
                                         conv_ps[g][:, j * 128:(j + 1) * 128],
                                         AF.Relu, bias=cbias_sb[:, m:m + 1])

                if upto < 2:
                    continue
                # ---- input proj: stA[0:8] = relu(feat @ ipw + ipb) ----
                ip_ps = [psp.tile([128, 512], dt.float32, tag="ps",
                                  name=f"ip_ps{g}") for g in range(2)]
                for k2 in range(CM // 2):
                    ipw_t = ipp.tile([128, 2, SEN], dt.bfloat16, tag="ipw")
                    nc.sync.dma_start(
                        out=ipw_t[:, :, :],
                        in_=ipw_d[k2 * 256:(k2 + 1) * 256, :].rearrange(
                            "(k p) c -> p k c", p=128))
                    for i in range(2):
                        k = 2 * k2 + i
                        for m in range(AKT):
                            g, j = divmod(m, 4)
                            nc.tensor.matmul(ip_ps[g][:, j * 128:(j + 1) * 128],
                                             ipw_t[:, i, m * 128:(m + 1) * 128],
                                             featT[:, k, :],
                                             start=(k == 0 and j == 0),
                                             stop=(k == CM - 1 and j == 3))
                for m in range(AKT):
                    g, j = divmod(m, 4)
                    nc.scalar.activation(stA[:, m, :],
                                         ip_ps[g][:, j * 128:(j + 1) * 128],
                                         AF.Relu, bias=ipb_sb[:, m:m + 1])

                if not wa_loads_done:
                    wa_loads_done = True
                    for k in range(AKT - 1):
                        nc.sync.dma_start(
                            out=wa8_sb[:, k, :], in_=wa8_d[k * 128:(k + 1) * 128, :])
                        nc.sync.dma_start(
                            out=wao_sb[:, k, :], in_=wao_d[k * 128:(k + 1) * 128, :])
                    kl = AKT - 1
                    for h in range(2):
                        nc.sync.dma_start(
                            out=wa8_sb[:, kl, h * 1536:(h + 1) * 1536],
                            in_=wa8_d[kl * 128:(kl + 1) * 128,
                                      h * 1536:(h + 1) * 1536])
                    for h in range(2):
                        nc.sync.dma_start(
                            out=wao_sb[:, kl, h * 512:(h + 1) * 512],
                            in_=wao_d[kl * 128:(kl + 1) * 128,
                                      h * 512:(h + 1) * 512])

                if upto < 3:
                    continue
                # ---- t1: stB[m] = relu(sum_k wa[k, m] stA[k]), k in 0:8 ----
                t1_ps = [psp.tile([128, 512], dt.float32, tag="ps",
                                  name=f"t1_ps{g}") for g in range(8)]
                for k in range(AKT):
                    for m in range(NM):
                        g, j = divmod(m, 4)
                        nc.tensor.matmul(t1_ps[g][:, j * 128:(j + 1) * 128],
                                         _wa_stat(wa8_sb, wao_sb, k, m),
                                         stA[:, k, :],
                                         start=(k == 0 and j == 0),
                                         stop=(k == AKT - 1 and j == 3))
                for g in range(8):
                    nc.scalar.activation(stB[:, 4 * g:4 * g + 4, :], t1_ps[g][:],
                                         AF.Relu)



                # ---- t2, t3: full timesteps; src/dst state ping-pong ----
                # DoubleRow fp8 stream matmuls interleaved with the resident
                # A/B-part matmuls as PE filler, so the ws stream (2.37us/pair
                # DMA vs 0.65us/pair PE) hides behind resident compute.
                for t in (2, 3):
                    if upto < t + 2:
                        continue
                    src = stB if t == 2 else stA
                    dst = stA if t == 2 else stB
                    ps = [psp.tile([128, 512], dt.float32, tag="ps",
                                   name=f"t{t}_ps{g}") for g in range(8)]
                    st8 = s8p.tile([128, SKT // 2, 2, BSH], dt.float8e4,
                                   tag="st8")
                    for k in range(SKT):
                        kp, i = divmod(k, 2)
                        nc.vector.tensor_scalar_max(st8[:, kp, i, :],
                                                    src[:, AKT + k, :], 0.0)
                    st8a = s8p.tile([128, AKT // 2, 2, BSH], dt.float8e4,
                                    tag="st8a")
                    for k in range(AKT):
                        kp, i = divmod(k, 2)
                        nc.vector.tensor_scalar_max(st8a[:, kp, i, :],
                                                    src[:, k, :], 0.0)

                    def _a_dr(kp):
                        for m in range(OM0):
                            g, j = divmod(m, 4)
                            nc.tensor.matmul(
                                ps[g][:, j * 128:(j + 1) * 128],
                                wa8_sb[:, 2 * kp:2 * kp + 2,
                                       m * 128:(m + 1) * 128],
                                st8a[:, kp, :, :],
                                start=(kp == 0 and j == 0), stop=False,
                                perf_mode=mybir.MatmulPerfMode.DoubleRow)

                    def _a_o(k):
                        for m in range(OM0, NM):
                            g, j = divmod(m, 4)
                            nc.tensor.matmul(ps[g][:, j * 128:(j + 1) * 128],
                                             wao_sb[:, k, (m - OM0) * 128:
                                                    (m - OM0 + 1) * 128],
                                             src[:, k, :],
                                             start=(k == 0 and j == 0),
                                             stop=False)

                    def _b_part(k):
                        for m in range(OM0, NM):
                            g, j = divmod(m, 4)
                            nc.tensor.matmul(ps[g][:, j * 128:(j + 1) * 128],
                                             wb_sb[:, k, (m - OM0) * 128:
                                                   (m - OM0 + 1) * 128],
                                             src[:, AKT + k, :],
                                             start=False,
                                             stop=(k == SKT - 1 and m % 4 == 3))

                    def _dr_pair(kp, ws_t):
                        for m in range(OM0):
                            g, j = divmod(m, 4)
                            nc.tensor.matmul(
                                ps[g][:, j * 128:(j + 1) * 128],
                                ws_t[:, :, m * 128:(m + 1) * 128],
                                st8[:, kp, :, :],
                                start=False,
                                stop=(kp == SKT // 2 - 1 and m % 4 == 3),
                                perf_mode=mybir.MatmulPerfMode.DoubleRow)

                    # fillers: A-DR pairs, A-O chunks, then B k-tiles
                    fillers = ([lambda kp=kp: _a_dr(kp) for kp in (1, 2, 3)]
                               + [lambda k=k: _a_o(k) for k in range(1, AKT)]
                               + [lambda k=k: _b_part(k) for k in range(SKT)])
                    sched = (1, 1, 1, 2, 2, 2, 2, 4, 4, 4, 5, 6)
                    _a_dr(0)
                    _a_o(0)
                    fi = 0
                    for kp in range(SKT // 2):
                        if kp < 2:
                            cached = ws_c if kp == 0 else ws_c1
                            if t == 2 and rep == 0:
                                nc.sync.dma_start(
                                    out=cached[:, :, :],
                                    in_=ws_d[kp * 256:(kp + 1) * 256, :].rearrange(
                                        "(i p) c -> p i c", p=128))
                            ws_t = cached
                        else:
                            ws_t = wsp.tile([128, 2, SCOL], dt.float8e4,
                                            tag="ws", name=f"ws_t{t}_{kp}")
                            nc.sync.dma_start(
                                out=ws_t[:, :, :],
                                in_=ws_d[kp * 256:(kp + 1) * 256, :].rearrange(
                                    "(i p) c -> p i c", p=128))
                        if t == 2 and rep == 0 and 4 <= kp <= 8:
                            for kb in range((kp - 4) * 5,
                                            min((kp - 4) * 5 + 5, SKT)):
                                nc.sync.dma_start(
                                    out=wb_sb[:, kb, :],
                                    in_=wb_d[kb * 128:(kb + 1) * 128, :])
                        _dr_pair(kp, ws_t)
                        for _ in range(sched[kp]):
                            if fi < len(fillers):
                                fillers[fi]()
                                fi += 1
                    while fi < len(fillers):
                        fillers[fi]()
                        fi += 1
                    for g in (6, 7):
                        nc.scalar.activation(dst[:, 4 * g:4 * g + 4, :], ps[g][:],
                                             AF.Relu)
                    for g in range(6):
                        nc.scalar.activation(dst[:, 4 * g:4 * g + 4, :], ps[g][:],
                                             AF.Relu)

                outw_sb = owp.tile([128, AKT, NUM_PAD], dt.bfloat16, tag="outw")
                for k in range(AKT):
                    nc.gpsimd.dma_start(out=outw_sb[:, k, :],
                                        in_=outw_d[k * 128:(k + 1) * 128, :])

                if upto < 6:
                    continue
                # ---- t4: O-chunks only -> stA[24:32] ----
                def _t4_chain(g):
                    t4_ps = psp.tile([128, 512], dt.float32, tag="ps",
                                     name=f"t4_ps{g}")
                    for k in range(AKT):
                        for j in range(4):
                            mo = 4 * g + j
                            nc.tensor.matmul(t4_ps[:, j * 128:(j + 1) * 128],
                                             wao_sb[:, k, mo * 128:(mo + 1) * 128],
                                             stB[:, k, :],
                                             start=(k == 0 and j == 0), stop=False)
                    for k in range(SKT):
                        for j in range(4):
                            mo = 4 * g + j
                            nc.tensor.matmul(t4_ps[:, j * 128:(j + 1) * 128],
                                             wb_sb[:, k, mo * 128:(mo + 1) * 128],
                                             stB[:, AKT + k, :],
                                             start=False,
                                             stop=(k == SKT - 1 and j == 3))
                    nc.scalar.activation(stA[:, OM0 + 4 * g:OM0 + 4 * g + 4, :],
                                         t4_ps[:], AF.Relu)

                if upto < 7:
                    continue
                # ---- output projection: outT[m] = outw[:,m].T @ O_state + ob ----
                chains = []
                m0 = 0
                for nm_q in (4, 4, 4, 3, 1):
                    out_ps = psp.tile([128, 512], dt.float32, tag="ps",
                                      name=f"out_ps{m0}")
                    chains.append((m0, nm_q, out_ps))
                    m0 += nm_q

                def _out_ksec(ksec):
                    for m0, nm_q, out_ps in chains:
                        for k in ksec:
                            for mi in range(nm_q):
                                nc.tensor.matmul(
                                    out_ps[:, mi * 128:(mi + 1) * 128],
                                    outw_sb[:, k, (m0 + mi) * 128:
                                            (m0 + mi + 1) * 128],
                                    stA[:, OM0 + k, :],
                                    start=(k == 0 and mi == 0),
                                    stop=(k == AKT - 1 and mi == nm_q - 1))

                _t4_chain(0)
                _t4_chain(1)
                _out_ksec(range(AKT))
                for m0, nm_q, out_ps in chains:
                    for mi in range(nm_q):
                        nc.vector.tensor_scalar_add(
                            ostage[:, m0 + mi, :],
                            out_ps[:, mi * 128:(mi + 1) * 128],
                            ob_sb[:, m0 + mi:m0 + mi + 1])
                    nc.sync.dma_start(
                        out=outT_d.rearrange("(m p) b -> p m b",
                                             p=128)[:, m0:m0 + nm_q, :],
                        in_=ostage[:, m0:m0 + nm_q, :])

    nc.compile()
    return nc


_PROGRAM_CACHE: dict = {}


def get_program(reps: int = 1, use_cc: bool = True):
    key = reps
    if key not in _PROGRAM_CACHE:
        _PROGRAM_CACHE[key] = _build_program(reps)
    return _PROGRAM_CACHE[key]


def _assemble_wbig(inputs):
    wbig = np.zeros((512, CNN_PAD), np.float32)
    cbias = np.zeros(CNN_PAD, np.float32)
    off = 0
    for k in range(1, 9):
        o = HW - k + 1
        w = np.asarray(inputs[f"conv_w{k}"], np.float32)
        cb = np.asarray(inputs["conv_b"], np.float32)[k - 1]
        py = np.arange(o)[:, None, None]
        px = np.arange(o)[None, :, None]
        cc = np.arange(C_IN)[None, None, :]
        ncol = np.arange(FN)[:, None, None]
        cols = off + ncol * o * o + py[None, :, :, 0] * o + px[None, :, :, 0]
        for dy in range(k):
            for dx in range(k):
                rows = (py + dy) * 64 + (px + dx) * 8 + cc
                wbig[rows[None, :, :, :], cols[:, :, :, None]] = \
                    w[:, :, dy, dx][:, None, None, :]
        cbias[off + np.arange(FN * o * o)] = np.repeat(cb, o * o)
        off += FN * o * o
    return wbig, cbias


def _vec128(v, cols):
    """Pack a [128*cols] vector as [128, cols] partition-major."""
    out = np.zeros((128, cols), np.float32)
    out[:] = np.asarray(v, np.float32).reshape(cols, 128).T
    return np.ascontiguousarray(out)


def _prep_inputs(inputs):
    x = np.asarray(inputs["x"], np.float32)
    W = np.asarray(inputs["W"], np.float32)
    lora_A = np.asarray(inputs["lora_A"], np.float32)
    lora_B = np.asarray(inputs["lora_B"], np.float32)
    ip_w = np.asarray(inputs["ip_w"], np.float32)
    ip_b = np.asarray(inputs["ip_b"], np.float32)
    out_w = np.asarray(inputs["out_w"], np.float32)
    out_b = np.asarray(inputs["out_b"], np.float32)

    wbig, cbias = _assemble_wbig(inputs)
    ipw_pad = np.zeros((CNN_PAD, SEN), np.float32)
    ipw_pad[:CNN_OUT] = ip_w
    oww_pad = np.zeros((OUT, NUM_PAD), np.float32)
    oww_pad[:, :NUM_OUT] = out_w
    ob_pad = np.zeros(NUM_PAD, np.float32)
    ob_pad[:NUM_OUT] = out_b

    mask = (W != 0).astype(np.float32)
    W1 = (W + (lora_A @ lora_B) * LORA_SCALE) * mask + np.eye(TOT, dtype=np.float32)

    def bf(a):
        return np.ascontiguousarray(a).astype(BF16)

    shared = {
        "wbig": bf(wbig),
        "cbias": _vec128(cbias, CM),
        "ipw": bf(ipw_pad),
        "ipb": _vec128(ip_b, AKT),
        "wa8": np.ascontiguousarray(W1[:SEN, :SCOL]).astype(FP8),
        "wao": bf(W1[:SEN, SCOL:]),
        "wb": bf(W1[SEN:, SCOL:]),
        "ws": np.ascontiguousarray(W1[SEN:, :SCOL]).astype(FP8),
        "outw": bf(oww_pad),
        "ob": _vec128(ob_pad, NO),
    }
    in_maps = []
    for c in range(N_CORES):
        xs = x[c * BSH:(c + 1) * BSH].reshape(BSH, 512).T
        m = dict(shared)
        m["xT"] = bf(xs)
        in_maps.append(m)
    return in_maps


def run_on_hw(in_maps, reps: int = 1):
    nc = get_program(reps)
    return run_bass_kernel_spmd(nc, in_maps, list(range(N_CORES)), trace=False)


def kernel(**inputs) -> np.ndarray:
    in_maps = _prep_inputs(inputs)
    res = run_on_hw(in_maps, reps=1)
    out = np.zeros((B, NUM_PAD), np.float32)
    for c in range(N_CORES):
        out[c * BSH:(c + 1) * BSH, :] = \
            np.asarray(res.results[c]["outT"]).astype(np.float32).T
    return np.ascontiguousarray(out[:, :NUM_OUT])
